# revision 55
# baseline (speedup 1.0000x reference)
"""Bahdanau attention scorer for Trainium2, 8-core data-parallel over batch.

scores[b, s] = v_a . tanh(W_s @ enc_outs[s, b] + W_t @ dec_out[b] + b_t)

Shapes (fixed): enc_outs (2048, 64, 512) f32, dec_out (64, 512) f32,
W_s/W_t (512, 512) f32, b_t/v_a (512,) f32 -> scores (64, 2048) f32.

Sharding: batch 64 -> 8 cores x 8 batches. Small params replicated.

Host prep does all layout work so the device kernel is pure streaming:
  * enc is pre-transposed per core to [block][hc][128 h][512 tokens] bf16
    so matmul contraction (over h) needs no on-device transposes.
  * dec bias (W_t @ dec + b_t) is computed host-side in f64 -> f32.
Per-core device pipeline, one (b, s-block) tile of 512 tokens per step:
  1. DMA HBM -> SBUF: per-h-chunk loads for the first four blocks (so
     fill-phase matmuls start before each block is resident, interleaved
     with the W_s chunk loads), then one whole-block DMA per block
     (contiguous 4 KB rows, a fifth of the SP descriptor-issue work).
  2. PE: 4x4 matmuls psum[ac] (128 a, 512 tok) += W_sT[hc,ac].T @ x[hc].
  3. ACT: tanh(psum + bias[b, ac]) -> bf16.
  4. DVE: scale each chunk by v_a, summing incrementally as chunks land.
  5. PE ones-matmul reduces 128 partitions -> DVE copy -> DMA out. The
     stationary is a full [128, 128] ones matrix: a [1, n] matmul output
     streams ~40% slower on HW than [128, n], so the reduce writes a full
     PSUM tile (every row the sum) and the copy takes row 0.
Scheduling: the partition-reduce matmul for block k is emitted after
block k+2's main matmuls so the PE never stalls on the ACT/DVE chain; a
dummy-matmul warm-up during the initial DMA fill pre-ramps the PE clock;
and the final block is split 384/128 with the second-to-last block
sandwiched between so the tail reduces always have matmul cover.
Measured: every matmul at the 216 ns full-clock floor (p90), PE busy
~93% of span, within ~8% of the 8-core bf16 PE roofline for this GEMM.
"""

import sys

sys.path.insert(0, "/opt/trn_rl_repo")

import numpy as np
import ml_dtypes

import concourse.bass as bass
import concourse.mybir as mybir
import concourse.tile as tile
from concourse import bacc
from concourse.bass_utils import run_bass_kernel_spmd

S, B, H, A = 2048, 64, 512, 512
NCORES = 8
BL = B // NCORES          # local batches per core
HC = H // 128             # h chunks
AC = A // 128             # a chunks
SBLK = 512                # tokens per block
NSB = S // SBLK           # s blocks per batch row
NBLK = BL * NSB           # blocks per core

F32 = mybir.dt.float32
BF16 = mybir.dt.bfloat16
BF16_NP = ml_dtypes.bfloat16

_CACHE = {}


def build_kernel():
    nc = bacc.Bacc("TRN2", target_bir_lowering=False, debug=False,
                   num_devices=NCORES)

    enc_d = nc.dram_tensor("enc", [NBLK * 128, HC * SBLK], BF16,
                           kind="ExternalInput")
    wst_d = nc.dram_tensor("wst", [128, HC * A], BF16, kind="ExternalInput")
    bias_d = nc.dram_tensor("bias", [128, AC * BL], F32, kind="ExternalInput")
    va_d = nc.dram_tensor("va", [128, AC], F32, kind="ExternalInput")
    out_d = nc.dram_tensor("scores", [1, BL * S], F32, kind="ExternalOutput")

    with tile.TileContext(nc) as tc:
        with tc.tile_pool(name="consts", bufs=1) as constp:
            wst_sb = constp.tile([128, HC * A], BF16, tag="wst")
            va_sb = constp.tile([128, AC], F32, tag="va")
            bias_sb = constp.tile([128, AC * BL], F32, tag="bias")
            # 128-wide all-ones stationary: the partition-reduce matmul then
            # writes a full [128, n] PSUM tile (every row the same sum),
            # which streams faster than a [1, n] output on HW
            ones_sb = constp.tile([128, 128], BF16, tag="ones")
            nc.gpsimd.memset(ones_sb[:], 1.0)
            warm_sb = constp.tile([128, 256], BF16, tag="warm")
            nc.gpsimd.memset(warm_sb[:], 1.0)

            with (
                tc.tile_pool(name="xin0", bufs=2 * HC) as xinp0,
                tc.tile_pool(name="xin", bufs=3) as xinp,
                tc.tile_pool(name="act", bufs=5 * AC) as actp,
                tc.tile_pool(name="stage", bufs=6) as stagep,
                tc.tile_pool(name="ps_mm", bufs=6, space="PSUM") as mmp,
                tc.tile_pool(name="ps_v", bufs=2, space="PSUM") as pvp,
            ):
                pending = []  # (vm_tile, b, sb, c0, w) awaiting reduce

                # warm-up: the PE p-state ramps only while the engine is
                # busy, and the first ~10 us are DMA fill where it idles.
                # Dummy matmuls during the fill pre-ramp the clock so the
                # first real matmuls run near full speed.
                for _ in range(13):
                    wps = pvp.tile([128, SBLK], F32, tag="pv")
                    nc.tensor.matmul(wps[:, 0:256], ones_sb[:], warm_sb[:],
                                     start=True, stop=True)

                def emit_reduce(vm, b, sb, c0, w):
                    psV = pvp.tile([128, SBLK], F32, tag="pv")
                    nc.tensor.matmul(psV[:, 0:w], ones_sb[:], vm[:, 0:w],
                                     start=True, stop=True)
                    stg = stagep.tile([1, SBLK], F32, tag="stage")
                    nc.vector.tensor_copy(stg[:, 0:w], psV[0:1, 0:w])
                    nc.sync.dma_start(
                        out_d[0:1, b * S + sb * SBLK + c0:
                              b * S + sb * SBLK + c0 + w], stg[:, 0:w])

                def emit_postmm(psM, b, sb, c0, w):
                    """tanh + v_a scale + incremental chunk-sum for columns
                    [c0, c0+w). Summing as chunks arrive keeps the post-mul
                    tail to a single DVE add."""
                    vacc = None
                    for ac in range(AC):
                        th = actp.tile([128, SBLK], BF16, tag="tanh")
                        nc.scalar.activation(
                            th[:, 0:w], psM[ac][:, 0:w],
                            mybir.ActivationFunctionType.Tanh,
                            bias=bias_sb[:, ac * BL + b: ac * BL + b + 1],
                        )
                        vm = actp.tile([128, SBLK], BF16, tag="vm")
                        nc.vector.tensor_scalar_mul(
                            vm[:, 0:w], th[:, 0:w], va_sb[:, ac:ac + 1])
                        if vacc is None:
                            vacc = vm
                        else:
                            nc.vector.tensor_add(vacc[:, 0:w], vacc[:, 0:w],
                                                 vm[:, 0:w])
                    pending.append((vacc, b, sb, c0, w))

                # the final block is split 384+128 so the very last reduce
                # only waits on a short 128-column ACT/DVE tail; the full
                # second-to-last block is sandwiched between the halves so
                # every tail reduce has matmul cover while its chain drains
                segments = [(blk, 0, SBLK) for blk in range(NBLK - 2)]
                segments += [(NBLK - 1, 0, 384), (NBLK - 2, 0, SBLK),
                             (NBLK - 1, 384, 128)]

                for blk, c0, w in segments:
                    b, sb = divmod(blk, NSB)
                    r0 = blk * 128
                    if blk < 4 and c0 == 0:
                        # fill phase: per-chunk loads into separate tiles so
                        # matmuls start before each block is fully resident.
                        # DMA descriptor-issue costs ~610 ns each on the
                        # issuing sequencer, so alternate blocks between the
                        # SP and ACT HWDGE queues (ACT idles during fill):
                        # block 0's data loads then issue concurrently with
                        # the weight-chunk loads instead of serially after.
                        eng = nc.scalar if blk % 2 == 0 else nc.sync
                        xc = []
                        for hc in range(HC):
                            if blk == 0:
                                nc.sync.dma_start(
                                    wst_sb[:, hc * A:(hc + 1) * A],
                                    wst_d[:, hc * A:(hc + 1) * A])
                            t = xinp0.tile([128, SBLK], BF16, tag=f"x{hc}")
                            eng.dma_start(
                                t[:],
                                enc_d[r0:r0 + 128,
                                      hc * SBLK:(hc + 1) * SBLK])
                            xc.append(t)
                        if blk == 0:
                            nc.sync.dma_start(va_sb[:], va_d[:])
                            nc.sync.dma_start(bias_sb[:], bias_d[:])
                        rhs = [xc[hc][:] for hc in range(HC)]
                    else:
                        # steady state: one whole-block DMA (contiguous 4 KB
                        # rows) — a fifth of the SP descriptor-issue work
                        xt = xinp.tile([128, HC * SBLK], BF16, tag="xt")
                        if w == SBLK:
                            nc.sync.dma_start(xt[:], enc_d[r0:r0 + 128, :])
                        else:
                            for hc in range(HC):
                                nc.sync.dma_start(
                                    xt[:, hc * SBLK + c0:hc * SBLK + c0 + w],
                                    enc_d[r0:r0 + 128,
                                          hc * SBLK + c0:hc * SBLK + c0 + w])
                        rhs = [xt[:, hc * SBLK + c0:hc * SBLK + c0 + w]
                               for hc in range(HC)]

                    psM = []
                    for ac in range(AC):
                        ps = mmp.tile([128, SBLK], F32, tag="mm")
                        psM.append(ps)
                    if blk < 4:
                        # fill phase, hc-outer: run all four ac passes on an
                        # already-arrived x chunk before needing the next
                        # one, with warm-up matmuls between block 0's groups
                        # so supply gaps don't reset the PE clock ramp
                        for hc in range(HC):
                            for ac in range(AC):
                                nc.tensor.matmul(
                                    psM[ac][:, 0:w],
                                    wst_sb[:, hc * A + ac * 128:
                                           hc * A + ac * 128 + 128],
                                    rhs[hc],
                                    start=(hc == 0), stop=(hc == HC - 1),
                                )
                            if blk == 0 and hc < HC - 1:
                                for _ in range(2):
                                    wps = pvp.tile([128, SBLK], F32,
                                                   tag="pv")
                                    nc.tensor.matmul(
                                        wps[:, 0:256], ones_sb[:],
                                        warm_sb[:], start=True, stop=True)
                    else:
                        for ac in range(AC):
                            for hc in range(HC):
                                nc.tensor.matmul(
                                    psM[ac][:, 0:w],
                                    wst_sb[:, hc * A + ac * 128:
                                           hc * A + ac * 128 + 128],
                                    rhs[hc],
                                    start=(hc == 0), stop=(hc == HC - 1),
                                )

                    # reduce for an earlier block now that two more blocks'
                    # matmuls are queued ahead of it on the PE — by then its
                    # ACT/DVE chain has certainly drained, so no PE stall
                    while len(pending) >= 2:
                        emit_reduce(*pending.pop(0))

                    emit_postmm(psM, b, sb, c0, w)

                # the flush reduces wait on the last ACT/DVE chains; keep
                # the PE clock ramp hot through those known gaps with
                # warm-ups on ONE dedicated tile (WAW-only — rotating pool
                # tiles here would couple the warm-ups to the DVE backlog)
                wps = pvp.tile([128, SBLK], F32, tag="pv")
                for p in pending:
                    for _ in range(4):
                        nc.tensor.matmul(wps[:, 0:256], ones_sb[:],
                                         warm_sb[:], start=True, stop=True)
                    emit_reduce(*p)

    nc.compile()
    return nc


def _prep_host(dec_out, enc_outs, W_s, W_t, b_t, v_a):
    # W_s.T laid out as [128 h-part, HC * A]
    wst = np.ascontiguousarray(
        W_s.T.reshape(HC, 128, A).transpose(1, 0, 2).reshape(128, HC * A)
    ).astype(BF16_NP)
    # dec bias, exact on host: bias[a, b] = (W_t @ dec[b] + b_t)[a]
    bias = (dec_out.astype(np.float64) @ W_t.T.astype(np.float64)
            + b_t.astype(np.float64)).T.astype(np.float32)   # (A, B)
    va4 = np.ascontiguousarray(
        v_a.reshape(AC, 128).T).astype(np.float32)           # (128, AC)

    enc_bf = enc_outs.astype(BF16_NP)                        # (S, B, H)
    in_maps = []
    for k in range(NCORES):
        # -> [b, sb, p, hc, c] -> row (b*NSB+sb)*128 + p, col hc*SBLK + c
        e = enc_bf[:, k * BL:(k + 1) * BL, :]
        e6 = e.reshape(NSB, SBLK, BL, HC, 128).transpose(2, 0, 4, 3, 1)
        enc_l = np.ascontiguousarray(e6).reshape(NBLK * 128, HC * SBLK)
        bl = bias[:, k * BL:(k + 1) * BL]                    # (A, BL)
        bias_l = np.ascontiguousarray(
            bl.reshape(AC, 128, BL).transpose(1, 0, 2).reshape(128, AC * BL))
        in_maps.append({
            "enc": enc_l,
            "wst": wst,
            "bias": bias_l,
            "va": va4,
        })
    return in_maps


def kernel(dec_out, enc_outs, W_s, W_t, b_t, v_a, trace=False):
    dec_out = np.asarray(dec_out)
    enc_outs = np.asarray(enc_outs)
    if "nc" not in _CACHE:
        _CACHE["nc"] = build_kernel()
    nc = _CACHE["nc"]
    in_maps = _prep_host(dec_out, enc_outs,
                         np.asarray(W_s), np.asarray(W_t),
                         np.asarray(b_t), np.asarray(v_a))
    res = run_bass_kernel_spmd(nc, in_maps, core_ids=list(range(NCORES)),
                               trace=trace)
    out = np.concatenate(
        [res.results[k]["scores"].reshape(BL, S) for k in range(NCORES)],
        axis=0).astype(np.float32)
    if trace:
        _CACHE["last_result"] = res
    return out
